# revision 62
# baseline (speedup 1.0000x reference)
"""Trainium2 Bass kernel for nn_Align_MoE_9732395892816 (moe_routing).

Strategy: data-parallel over the 4096 tokens across 8 NeuronCores (512
tokens/core == one batch row/core); every core holds the full expert stacks.

Per-core device kernel (feature-major activations, tokens on the free axis):
  - gates computed expert-major [E, T] in full fp32 (so top-2 selections
    match the fp32 reference bit-for-bit): Wg/Wf as stationary operands,
    exp+bias fused on ScalarE, softmax denominator via an all-ones PE
    matmul, top-2 sparsify via two gpsimd partition_all_reduce maxes +
    threshold; the `weight` scalars are folded into the routing rows
  - expert matmuls run in bf16 (weights, x, hidden) with fp32 PSUM
    accumulation: halves the weight DMA stream (134MB vs 268MB) and the
    LDWEIGHTS time so stationary loads hide behind the matmul stream
  - per expert e: hidden = relu(x @ W1[e] + b1[e]) with PSUM accumulation
    over the 16 k-tiles; ScalarE writes the Relu output directly as bf16
  - second matmul per output d-tile, then post-scale by the broadcast
    routing row (PE one-hot broadcast) and accumulate into SBUF; b2 enters
    via a routeT @ b2 matmul that initializes the accumulators
  - phase A of expert 0 is issued before the bias-init/broadcast matmuls
    so the PE does not idle while gpsimd finishes the top-2 thresholding
  - outputs are written feature-major [1024, 512]; the host transposes back

kernel(**inputs) marshals the full inputs, runs the SPMD NEFF on cores 0-7,
and reassembles the full (out0, out1) tuple exactly like the reference.
"""

import os
import sys

for _p in ("/opt/trn_rl_repo",):
    if _p not in sys.path:
        sys.path.insert(0, _p)

import numpy as np
import ml_dtypes

import concourse.mybir as mybir
import concourse.tile as tile
from concourse import bacc
from concourse.bass import ts
from concourse.bass_utils import run_bass_kernel_spmd
from concourse import bass_isa

F32 = mybir.dt.float32
BF16 = mybir.dt.bfloat16
P = 128

# problem sizes (hardcoded per spec)
B, S, D, E, TOPK = 8, 512, 2048, 8, 2
NCORES = 8
T = B * S // NCORES          # tokens per core
H = D // 2

LAST_EXEC_TIME_NS = None     # set when MOE_TRACE=1


def _build_moe(T, D, E, w_bufs=5, psum_bufs=3, psumb_bufs=4):
    """Build + bacc-compile the per-core module."""
    assert E == 8
    KT = D // P            # k-tiles over model dim (also h-tile count)
    H = D // 2
    HKo = H // P           # gate contraction k-tiles
    JT = KT                # output d-tiles (both halves)
    JH = JT // 2
    AF = mybir.ActivationFunctionType

    nc = bacc.Bacc()
    xt = nc.dram_tensor("xt", [D, T], F32, kind="ExternalInput")
    xb = nc.dram_tensor("xb", [D, T], BF16, kind="ExternalInput")
    w1r = nc.dram_tensor("w1r", [E, KT, P, KT, P], BF16, kind="ExternalInput")
    w2r = nc.dram_tensor("w2r", [E, JT, P, KT, P], BF16, kind="ExternalInput")
    wg = nc.dram_tensor("wg", [P, HKo, E], F32, kind="ExternalInput")
    wf = nc.dram_tensor("wf", [P, HKo, E], F32, kind="ExternalInput")
    bgt = nc.dram_tensor("bgt", [1, E], F32, kind="ExternalInput")
    bft = nc.dram_tensor("bft", [1, E], F32, kind="ExternalInput")
    b1r = nc.dram_tensor("b1r", [P, E, KT], F32, kind="ExternalInput")
    b2t = nc.dram_tensor("b2t", [E, D], BF16, kind="ExternalInput")
    wv = nc.dram_tensor("wv", [1, 2], F32, kind="ExternalInput")
    y0 = nc.dram_tensor("y0", [H, T], F32, kind="ExternalOutput")
    y1 = nc.dram_tensor("y1", [H, T], F32, kind="ExternalOutput")
    # DRAM staging for the routing rows: a 0-stride DMA read broadcasts a
    # single row across all 128 partitions without touching PE/DVE
    rgD = nc.dram_tensor("rgD", [E, T], BF16, kind="Internal")
    rfD = nc.dram_tensor("rfD", [E, T], BF16, kind="Internal")

    with tile.TileContext(nc) as tc:
        with (
            tc.tile_pool(name="const", bufs=1) as cpool,
            tc.tile_pool(name="wpool1", bufs=w_bufs) as w1pool,
            tc.tile_pool(name="wpool2", bufs=w_bufs + 1) as w2pool,
            tc.tile_pool(name="bcastp", bufs=2) as bpool,
            tc.tile_pool(name="accp", bufs=3) as apool,
            tc.tile_pool(name="psA", bufs=psumb_bufs, space="PSUM") as psumA,
            tc.tile_pool(name="psB", bufs=psum_bufs, space="PSUM") as psumB,
            tc.tile_pool(name="psC", bufs=1, space="PSUM") as psumC,
            tc.tile_pool(name="gsb", bufs=1) as gsb,
        ):
            # ---- persistent tiles ----
            # the bf16 x feeds the very first PE work (phase A of expert
            # 0): issue it as one wide DMA first so HW-DGE fans it out
            # DMA triggers cost ~650ns each on a sequencer and only SP
            # (sync) and Activation (scalar) have HW-DGE: interleave the
            # x tiles across both and keep the weight stream on sync so
            # trigger issue never serializes behind one engine
            # prefetch the first w1 tiles of expert 0 ahead of the x stream
            # so the first PE chains never wait on weight supply
            w1_pre = []
            for hk in range(4):
                w1t = w1pool.tile([P, KT, P], BF16, tag="w1t")
                nc.sync.dma_start(w1t[:], w1r[0, hk])
                w1_pre.append(w1t)
            XB = cpool.tile([P, KT, T], BF16)
            xb_r = xb.rearrange("(ko p) t -> p ko t", p=P)
            for ko in range(KT):
                eng = nc.scalar if ko % 2 == 0 else nc.sync
                eng.dma_start(XB[:, ko, :], xb_r[:, ko, :])
            wg_sb = cpool.tile([P, HKo, E], F32)
            nc.scalar.dma_start(wg_sb[:], wg[:])
            wf_sb = cpool.tile([P, HKo, E], F32)
            nc.scalar.dma_start(wf_sb[:], wf[:])
            bg8 = cpool.tile([E, 1], F32)
            nc.scalar.dma_start(bg8[:], bgt.rearrange("o e -> e o"))
            bf8 = cpool.tile([E, 1], F32)
            nc.scalar.dma_start(bf8[:], bft.rearrange("o e -> e o"))
            b1_sb = cpool.tile([P, E, KT], F32)
            nc.scalar.dma_start(b1_sb[:], b1r[:])
            b2_sb = cpool.tile([E, D], BF16)
            nc.scalar.dma_start(b2_sb[:], b2t[:])
            wv_sb = cpool.tile([1, 2], F32)
            nc.scalar.dma_start(wv_sb[:], wv[:])
            ones_sb = cpool.tile([1, P], F32)
            nc.vector.memset(ones_sb, 1.0)
            ones8 = cpool.tile([E, E], F32)
            nc.vector.memset(ones8, 1.0)
            routeTg = cpool.tile([E, T], BF16)
            routeTf = cpool.tile([E, T], BF16)
            out_sb = cpool.tile([P, JT, T], F32)
            htmp = cpool.tile([P, KT, T], BF16)

            # the fp32 x for the gate matmuls is issued mid-phase-A: early
            # enough for the gates (~70us in), late enough that its 4MB
            # doesn't jam the queues while phase A streams its w1 tiles
            XT = cpool.tile([P, KT, T], F32)
            xt_r = xt.rearrange("(ko p) t -> p ko t", p=P)

            def issue_xt():
                for ko in range(KT):
                    nc.scalar.dma_start(XT[:, ko, :], xt_r[:, ko, :])

            def phase_a(e):
                # hidden = relu(x @ W1[e] + b1[e]) -> htmp (feature-major)
                for hk in range(KT):
                    if e == 0 and hk == 6:
                        issue_xt()
                    if e == 0 and hk < len(w1_pre):
                        w1t = w1_pre[hk]
                    else:
                        w1t = w1pool.tile([P, KT, P], BF16, tag="w1t")
                        nc.sync.dma_start(w1t[:], w1r[e, hk])
                    psh = psumA.tile([P, T], F32, tag="psh")
                    for dk in range(KT):
                        nc.tensor.matmul(
                            psh,
                            lhsT=w1t[:, dk, :],
                            rhs=XB[:, dk, :],
                            start=(dk == 0),
                            stop=(dk == KT - 1),
                        )
                    nc.scalar.activation(htmp[:, hk, :], psh, AF.Relu,
                                         bias=b1_sb[:, e, hk:hk + 1])

            # expert 0 phase A is the first PE work: it only needs XB and
            # W1[0], so it starts as soon as those DMAs land; the gate
            # phase (which waits on the fp32 XT) overlaps with it
            phase_a(0)

            # ---- gate phase (expert-major), full fp32 ----
            wvb_ps = psumA.tile([P, 2], F32, tag="psh")
            nc.tensor.matmul(wvb_ps, ones_sb, wv_sb, start=True, stop=True)
            wvb = cpool.tile([P, 2], F32)
            nc.vector.tensor_copy(wvb, wvb_ps)

            RO = bass_isa.ReduceOp
            for which in ("g", "f"):
                w_sb = wg_sb if which == "g" else wf_sb
                bias8 = bg8 if which == "g" else bf8
                ko0 = 0 if which == "g" else HKo
                psg = psumC.tile([E, T], F32, tag="bps")
                for ko in range(HKo):
                    nc.tensor.matmul(psg,
                                     lhsT=w_sb[:, ko, :],
                                     rhs=XT[:, ko0 + ko, :],
                                     start=(ko == 0),
                                     stop=(ko == HKo - 1))
                # exp(logit + bias); logits are O(1) so no max-subtraction
                exv = gsb.tile([E, T], F32, tag="gb")
                nc.scalar.activation(exv, psg, AF.Exp, bias=bias8[:, 0:1])
                # softmax denominator via a tiny all-ones matmul (all
                # partitions get the partition-sum)
                pss = psumC.tile([E, T], F32, tag="bps")
                nc.tensor.matmul(pss, lhsT=ones8[:, :], rhs=exv[:, :],
                                 start=True, stop=True)
                rcp = gsb.tile([E, T], F32, tag="gc")
                nc.vector.reciprocal(rcp, pss)
                rout = gsb.tile([E, T], F32, tag="ga")
                nc.vector.tensor_mul(rout, exv, rcp)
                if which == "g":
                    # top-2 sparsify: zero entries below the 2nd-largest prob
                    mx1 = gsb.tile([E, T], F32, tag="red")
                    nc.gpsimd.partition_all_reduce(mx1[:], rout[:], channels=E,
                                                   reduce_op=RO.max)
                    msk = gsb.tile([E, T], F32, tag="gb2")
                    nc.vector.tensor_tensor(msk, rout, mx1,
                                            mybir.AluOpType.is_ge)
                    nc.vector.tensor_scalar_mul(msk, msk, 1e30)
                    nc.vector.tensor_sub(msk, rout, msk)
                    mx2 = gsb.tile([E, T], F32, tag="red2")
                    nc.gpsimd.partition_all_reduce(mx2[:], msk[:], channels=E,
                                                   reduce_op=RO.max)
                    keep = gsb.tile([E, T], F32, tag="gc2")
                    nc.vector.tensor_tensor(keep, rout, mx2,
                                            mybir.AluOpType.is_ge)
                    nc.vector.tensor_scalar_mul(keep, keep, wvb[0:E, 0:1])
                    nc.vector.tensor_mul(routeTg[:, :], rout, keep)
                else:
                    nc.vector.tensor_scalar_mul(routeTf[:, :], rout,
                                                wvb[0:E, 1:2])

            # staged after BOTH gate halves: a routeTg-dependent trigger
            # between them would stall the scalar sequencer before exp(f)
            nc.scalar.dma_start(rgD[:], routeTg[:, :])
            nc.scalar.dma_start(rfD[:], routeTf[:, :])

            # ---- expert loop ----
            # (bias init out_sb[j] = routeT @ b2 happens inside the e==0
            # j-loop so the PE flows from the gates straight into B(0)'s
            # chains instead of stalling on the gpsimd top-2 result)
            for e in range(E):
                # broadcast routing rows across partitions via 0-stride
                # DMA reads of the DRAM staging copy (off every engine)
                bgb = bpool.tile([P, T], BF16, tag="bgb")
                bfb = bpool.tile([P, T], BF16, tag="bfb")
                nc.scalar.dma_start(
                    bgb[:], rgD[e:e + 1, :].partition_broadcast(P).squeeze(1))
                nc.scalar.dma_start(
                    bfb[:], rfD[e:e + 1, :].partition_broadcast(P).squeeze(1))

                if e > 0:
                    phase_a(e)

                # Phase B: out_j += route[e] * (hidden @ W2[e] chunk)
                for j in range(JT):
                    w2t = w2pool.tile([P, KT, P], BF16, tag="w2t")
                    nc.sync.dma_start(w2t[:], w2r[e, j])
                    pso = psumB.tile([P, T], F32, tag="pso")
                    for hk in range(KT):
                        nc.tensor.matmul(
                            pso,
                            lhsT=w2t[:, hk, :],
                            rhs=htmp[:, hk, :],
                            start=(hk == 0),
                            stop=(hk == KT - 1),
                        )
                    if e == 0:
                        routeT = routeTg if j < JH else routeTf
                        psb = psumC.tile([P, T], F32, tag="bps")
                        nc.tensor.matmul(psb, lhsT=b2_sb[:, ts(j, P)],
                                         rhs=routeT[:, :],
                                         start=True, stop=True)
                        nc.vector.tensor_copy(out_sb[:, j, :], psb)
                    bsrc = bgb if j < JH else bfb
                    tmp = apool.tile([P, T], F32, tag="acc")
                    nc.vector.tensor_mul(tmp, pso, bsrc)
                    nc.vector.tensor_add(out_sb[:, j, :], out_sb[:, j, :], tmp)
                    if e == E - 1:
                        # final value for this d-tile: stream it out now,
                        # split in two so the last store's transfer halves
                        yt = y0 if j < JH else y1
                        jj = j if j < JH else j - JH
                        for h2 in range(2):
                            nc.scalar.dma_start(
                                yt[ts(jj, P), ts(h2, T // 2)],
                                out_sb[:, j, ts(h2, T // 2)])

    nc.compile()
    return nc


_NC_CACHE = {}


def _get_nc():
    if "nc" not in _NC_CACHE:
        _NC_CACHE["nc"] = _build_moe(T, D, E)
    return _NC_CACHE["nc"]


def _fingerprint(*arrays):
    parts = []
    for a in arrays:
        a = np.asarray(a)
        flat = a.reshape(-1)
        step = max(1, flat.size // 64)
        parts.append((id(a), a.shape, flat[::step][:64].tobytes()))
    return hash(tuple((i, s, b) for i, s, b in parts))


def _prep_shared(Wg, bg, Wf, bf, W1, b1, W2, b2, weight):
    KT = D // P
    HKo = H // P
    f32 = np.float32
    bf16 = ml_dtypes.bfloat16
    return {
        "w1r": np.ascontiguousarray(
            W1.reshape(E, KT, P, KT, P).transpose(0, 3, 2, 1, 4)).astype(bf16),
        "w2r": np.ascontiguousarray(
            W2.reshape(E, KT, P, KT, P).transpose(0, 3, 2, 1, 4)).astype(bf16),
        "wg": np.ascontiguousarray(
            Wg.reshape(HKo, P, E).transpose(1, 0, 2)).astype(f32, copy=False),
        "wf": np.ascontiguousarray(
            Wf.reshape(HKo, P, E).transpose(1, 0, 2)).astype(f32, copy=False),
        "bgt": np.ascontiguousarray(np.asarray(bg, f32).reshape(1, E)),
        "bft": np.ascontiguousarray(np.asarray(bf, f32).reshape(1, E)),
        "b1r": np.ascontiguousarray(
            b1.reshape(E, KT, P).transpose(2, 0, 1)).astype(f32, copy=False),
        "b2t": np.ascontiguousarray(np.asarray(b2, f32)).astype(bf16),
        "wv": np.ascontiguousarray(np.asarray(weight, f32).reshape(1, 2)),
    }


def kernel(vector, Wg, bg, Wf, bf, W1, b1, W2, b2, weight, top_k):
    """Full inputs in, full output out (tuple (out0, out1), matching the
    reference)."""
    global LAST_EXEC_TIME_NS
    assert int(top_k) == TOPK, f"kernel compiled for top_k={TOPK}"
    vector = np.asarray(vector, np.float32)
    assert vector.shape == (B, S, D), vector.shape

    nc = _get_nc()
    fp = _fingerprint(Wg, bg, Wf, bf, W1, b1, W2, b2, weight)
    if _NC_CACHE.get("shared_fp") != fp:
        _NC_CACHE["shared"] = _prep_shared(
            np.asarray(Wg, np.float32), bg, np.asarray(Wf, np.float32), bf,
            np.asarray(W1, np.float32), np.asarray(b1, np.float32),
            np.asarray(W2, np.float32), np.asarray(b2, np.float32), weight)
        _NC_CACHE["shared_fp"] = fp
    shared = _NC_CACHE["shared"]

    tokens = vector.reshape(B * S, D)
    in_maps = []
    for c in range(NCORES):
        m = dict(shared)
        xtc = np.ascontiguousarray(tokens[c * T:(c + 1) * T].T)
        m["xt"] = xtc
        m["xb"] = xtc.astype(ml_dtypes.bfloat16)
        in_maps.append(m)

    trace = bool(os.environ.get("MOE_TRACE"))
    res = run_bass_kernel_spmd(nc, in_maps, core_ids=list(range(NCORES)),
                               trace=trace)
    if trace:
        LAST_EXEC_TIME_NS = res.exec_time_ns

    y0 = np.stack([res.results[c]["y0"].T for c in range(NCORES)])
    y1 = np.stack([res.results[c]["y1"].T for c in range(NCORES)])
    out0 = np.ascontiguousarray(y0.reshape(B, S, H))
    out1 = np.ascontiguousarray(y1.reshape(B, S, H))
    return (out0, out1)


# revision 65
# speedup vs baseline: 1.0251x; 1.0251x over previous
"""Trainium2 Bass kernel for nn_Align_MoE_9732395892816 (moe_routing).

Strategy: data-parallel over the 4096 tokens across 8 NeuronCores (512
tokens/core == one batch row/core); every core holds the full expert stacks.

Per-core device kernel (feature-major activations, tokens on the free axis):
  - gates computed expert-major [E, T] in full fp32 (so top-2 selections
    match the fp32 reference bit-for-bit): Wg/Wf as stationary operands,
    exp+bias fused on ScalarE, softmax denominator via an all-ones PE
    matmul, top-2 sparsify via two gpsimd partition_all_reduce maxes +
    threshold; the `weight` scalars are folded into the routing rows
  - expert matmuls run in bf16 (weights, x, hidden) with fp32 PSUM
    accumulation: halves the weight DMA stream (134MB vs 268MB) and the
    LDWEIGHTS time so stationary loads hide behind the matmul stream
  - per expert e: hidden = relu(x @ W1[e] + b1[e]) with PSUM accumulation
    over the 16 k-tiles; ScalarE writes the Relu output directly as bf16
  - second matmul per output d-tile, then post-scale by the broadcast
    routing row (PE one-hot broadcast) and accumulate into SBUF; b2 enters
    via a routeT @ b2 matmul that initializes the accumulators
  - phase A of expert 0 is issued before the bias-init/broadcast matmuls
    so the PE does not idle while gpsimd finishes the top-2 thresholding
  - outputs are written feature-major [1024, 512]; the host transposes back

kernel(**inputs) marshals the full inputs, runs the SPMD NEFF on cores 0-7,
and reassembles the full (out0, out1) tuple exactly like the reference.
"""

import os
import sys

for _p in ("/opt/trn_rl_repo",):
    if _p not in sys.path:
        sys.path.insert(0, _p)

import numpy as np
import ml_dtypes

import concourse.mybir as mybir
import concourse.tile as tile
from concourse import bacc
from concourse.bass import ts
from concourse.bass_utils import run_bass_kernel_spmd
from concourse import bass_isa

F32 = mybir.dt.float32
BF16 = mybir.dt.bfloat16
P = 128

# problem sizes (hardcoded per spec)
B, S, D, E, TOPK = 8, 512, 2048, 8, 2
NCORES = 8
T = B * S // NCORES          # tokens per core
H = D // 2

LAST_EXEC_TIME_NS = None     # set when MOE_TRACE=1


def _build_moe(T, D, E, w_bufs=5, psum_bufs=3, psumb_bufs=4):
    """Build + bacc-compile the per-core module."""
    assert E == 8
    KT = D // P            # k-tiles over model dim (also h-tile count)
    H = D // 2
    HKo = H // P           # gate contraction k-tiles
    JT = KT                # output d-tiles (both halves)
    JH = JT // 2
    AF = mybir.ActivationFunctionType

    nc = bacc.Bacc()
    xt = nc.dram_tensor("xt", [D, T], F32, kind="ExternalInput")
    xb = nc.dram_tensor("xb", [D, T], BF16, kind="ExternalInput")
    w1r = nc.dram_tensor("w1r", [E, KT, P, KT, P], BF16, kind="ExternalInput")
    w2r = nc.dram_tensor("w2r", [E, JT, P, KT, P], BF16, kind="ExternalInput")
    wg = nc.dram_tensor("wg", [P, HKo, E], F32, kind="ExternalInput")
    wf = nc.dram_tensor("wf", [P, HKo, E], F32, kind="ExternalInput")
    bgt = nc.dram_tensor("bgt", [1, E], F32, kind="ExternalInput")
    bft = nc.dram_tensor("bft", [1, E], F32, kind="ExternalInput")
    b1r = nc.dram_tensor("b1r", [P, E, KT], F32, kind="ExternalInput")
    b2t = nc.dram_tensor("b2t", [E, D], BF16, kind="ExternalInput")
    wv = nc.dram_tensor("wv", [1, 2], F32, kind="ExternalInput")
    y0 = nc.dram_tensor("y0", [H, T], F32, kind="ExternalOutput")
    y1 = nc.dram_tensor("y1", [H, T], F32, kind="ExternalOutput")
    # DRAM staging for the routing rows: a 0-stride DMA read broadcasts a
    # single row across all 128 partitions without touching PE/DVE
    rgD = nc.dram_tensor("rgD", [E, T], BF16, kind="Internal")
    rfD = nc.dram_tensor("rfD", [E, T], BF16, kind="Internal")

    with tile.TileContext(nc) as tc:
        with (
            tc.tile_pool(name="const", bufs=1) as cpool,
            tc.tile_pool(name="wpool1", bufs=w_bufs) as w1pool,
            tc.tile_pool(name="wpool2", bufs=w_bufs + 1) as w2pool,
            tc.tile_pool(name="bcastp", bufs=2) as bpool,
            tc.tile_pool(name="accp", bufs=3) as apool,
            tc.tile_pool(name="psA", bufs=psumb_bufs, space="PSUM") as psumA,
            tc.tile_pool(name="psB", bufs=psum_bufs, space="PSUM") as psumB,
            tc.tile_pool(name="psC", bufs=1, space="PSUM") as psumC,
            tc.tile_pool(name="gsb", bufs=1) as gsb,
        ):
            # ---- persistent tiles ----
            # the bf16 x feeds the very first PE work (phase A of expert
            # 0): issue it as one wide DMA first so HW-DGE fans it out
            # DMA triggers cost ~650ns each on a sequencer and only SP
            # (sync) and Activation (scalar) have HW-DGE: interleave the
            # x tiles across both and keep the weight stream on sync so
            # trigger issue never serializes behind one engine
            # prefetch the first w1 tiles of expert 0 ahead of the x stream
            # so the first PE chains never wait on weight supply
            w1_pre = []
            for hk in range(4):
                w1t = w1pool.tile([P, KT, P], BF16, tag="w1t")
                nc.sync.dma_start(w1t[:], w1r[0, hk])
                w1_pre.append(w1t)
            XB = cpool.tile([P, KT, T], BF16)
            xb_r = xb.rearrange("(ko p) t -> p ko t", p=P)
            for ko in range(KT):
                eng = nc.scalar if ko % 2 == 0 else nc.sync
                eng.dma_start(XB[:, ko, :], xb_r[:, ko, :])
            wg_sb = cpool.tile([P, HKo, E], F32)
            nc.scalar.dma_start(wg_sb[:], wg[:])
            wf_sb = cpool.tile([P, HKo, E], F32)
            nc.scalar.dma_start(wf_sb[:], wf[:])
            bg8 = cpool.tile([E, 1], F32)
            nc.scalar.dma_start(bg8[:], bgt.rearrange("o e -> e o"))
            bf8 = cpool.tile([E, 1], F32)
            nc.scalar.dma_start(bf8[:], bft.rearrange("o e -> e o"))
            b1_sb = cpool.tile([P, E, KT], F32)
            nc.scalar.dma_start(b1_sb[:], b1r[:])
            b2_sb = cpool.tile([E, D], BF16)
            nc.scalar.dma_start(b2_sb[:], b2t[:])
            wv_sb = cpool.tile([1, 2], F32)
            nc.scalar.dma_start(wv_sb[:], wv[:])
            ones_sb = cpool.tile([1, P], F32)
            nc.vector.memset(ones_sb, 1.0)
            ones8 = cpool.tile([E, E], F32)
            nc.vector.memset(ones8, 1.0)
            routeTg = cpool.tile([E, T], BF16)
            routeTf = cpool.tile([E, T], BF16)
            out_sb = cpool.tile([P, JT, T], F32)
            # double-buffered hidden: phase A of expert e+1 interleaves
            # with phase B of expert e on the PE without a storage hazard
            htmp = cpool.tile([P, 2, KT, T], BF16)

            # the fp32 x for the gate matmuls is issued mid-phase-A: early
            # enough for the gates (~70us in), late enough that its 4MB
            # doesn't jam the queues while phase A streams its w1 tiles
            XT = cpool.tile([P, KT, T], F32)
            xt_r = xt.rearrange("(ko p) t -> p ko t", p=P)

            def issue_xt():
                for ko in range(KT):
                    nc.scalar.dma_start(XT[:, ko, :], xt_r[:, ko, :])

            def a_chain(e, hk):
                # one hidden chain: htmp[e%2, hk] = relu(x @ W1[e,:,hk] + b1)
                if e == 0 and hk == 6:
                    issue_xt()
                if e == 0 and hk < len(w1_pre):
                    w1t = w1_pre[hk]
                else:
                    w1t = w1pool.tile([P, KT, P], BF16, tag="w1t")
                    nc.sync.dma_start(w1t[:], w1r[e, hk])
                psh = psumA.tile([P, T], F32, tag="psh")
                for dk in range(KT):
                    nc.tensor.matmul(
                        psh,
                        lhsT=w1t[:, dk, :],
                        rhs=XB[:, dk, :],
                        start=(dk == 0),
                        stop=(dk == KT - 1),
                    )
                nc.scalar.activation(htmp[:, e % 2, hk, :], psh, AF.Relu,
                                     bias=b1_sb[:, e, hk:hk + 1])

            def phase_a(e):
                for hk in range(KT):
                    a_chain(e, hk)

            # expert 0 phase A is the first PE work: it only needs XB and
            # W1[0], so it starts as soon as those DMAs land; the gate
            # phase (which waits on the fp32 XT) overlaps with it
            phase_a(0)

            # ---- gate phase (expert-major), full fp32 ----
            wvb_ps = psumA.tile([P, 2], F32, tag="psh")
            nc.tensor.matmul(wvb_ps, ones_sb, wv_sb, start=True, stop=True)
            wvb = cpool.tile([P, 2], F32)
            nc.vector.tensor_copy(wvb, wvb_ps)

            RO = bass_isa.ReduceOp
            for which in ("g", "f"):
                w_sb = wg_sb if which == "g" else wf_sb
                bias8 = bg8 if which == "g" else bf8
                ko0 = 0 if which == "g" else HKo
                psg = psumC.tile([E, T], F32, tag="bps")
                for ko in range(HKo):
                    nc.tensor.matmul(psg,
                                     lhsT=w_sb[:, ko, :],
                                     rhs=XT[:, ko0 + ko, :],
                                     start=(ko == 0),
                                     stop=(ko == HKo - 1))
                # exp(logit + bias); logits are O(1) so no max-subtraction
                exv = gsb.tile([E, T], F32, tag="gb")
                nc.scalar.activation(exv, psg, AF.Exp, bias=bias8[:, 0:1])
                # softmax denominator via a tiny all-ones matmul (all
                # partitions get the partition-sum)
                pss = psumC.tile([E, T], F32, tag="bps")
                nc.tensor.matmul(pss, lhsT=ones8[:, :], rhs=exv[:, :],
                                 start=True, stop=True)
                rcp = gsb.tile([E, T], F32, tag="gc")
                nc.vector.reciprocal(rcp, pss)
                rout = gsb.tile([E, T], F32, tag="ga")
                nc.vector.tensor_mul(rout, exv, rcp)
                if which == "g":
                    # top-2 sparsify: zero entries below the 2nd-largest prob
                    mx1 = gsb.tile([E, T], F32, tag="red")
                    nc.gpsimd.partition_all_reduce(mx1[:], rout[:], channels=E,
                                                   reduce_op=RO.max)
                    msk = gsb.tile([E, T], F32, tag="gb2")
                    nc.vector.tensor_tensor(msk, rout, mx1,
                                            mybir.AluOpType.is_ge)
                    nc.vector.tensor_scalar_mul(msk, msk, 1e30)
                    nc.vector.tensor_sub(msk, rout, msk)
                    mx2 = gsb.tile([E, T], F32, tag="red2")
                    nc.gpsimd.partition_all_reduce(mx2[:], msk[:], channels=E,
                                                   reduce_op=RO.max)
                    keep = gsb.tile([E, T], F32, tag="gc2")
                    nc.vector.tensor_tensor(keep, rout, mx2,
                                            mybir.AluOpType.is_ge)
                    nc.vector.tensor_scalar_mul(keep, keep, wvb[0:E, 0:1])
                    nc.vector.tensor_mul(routeTg[:, :], rout, keep)
                else:
                    nc.vector.tensor_scalar_mul(routeTf[:, :], rout,
                                                wvb[0:E, 1:2])

            # staged after BOTH gate halves: a routeTg-dependent trigger
            # between them would stall the scalar sequencer before exp(f)
            nc.scalar.dma_start(rgD[:], routeTg[:, :])
            nc.scalar.dma_start(rfD[:], routeTf[:, :])

            # ---- expert loop ----
            # (bias init out_sb[j] = routeT @ b2 happens inside the e==0
            # j-loop so the PE flows from the gates straight into B(0)'s
            # chains instead of stalling on the gpsimd top-2 result)
            for e in range(E):
                # broadcast routing rows across partitions via 0-stride
                # DMA reads of the DRAM staging copy (off every engine)
                bgb = bpool.tile([P, T], BF16, tag="bgb")
                bfb = bpool.tile([P, T], BF16, tag="bfb")
                nc.scalar.dma_start(
                    bgb[:], rgD[e:e + 1, :].partition_broadcast(P).squeeze(1))
                nc.scalar.dma_start(
                    bfb[:], rfD[e:e + 1, :].partition_broadcast(P).squeeze(1))

                # Phase B: out_j += route[e] * (hidden @ W2[e] chunk),
                # interleaved chain-by-chain with phase A of expert e+1 so
                # either stream's supply hiccups hide behind the other
                for j in range(JT):
                    w2t = w2pool.tile([P, KT, P], BF16, tag="w2t")
                    nc.sync.dma_start(w2t[:], w2r[e, j])
                    pso = psumB.tile([P, T], F32, tag="pso")
                    for hk in range(KT):
                        nc.tensor.matmul(
                            pso,
                            lhsT=w2t[:, hk, :],
                            rhs=htmp[:, e % 2, hk, :],
                            start=(hk == 0),
                            stop=(hk == KT - 1),
                        )
                    if e + 1 < E:
                        a_chain(e + 1, j)
                    if e == 0:
                        routeT = routeTg if j < JH else routeTf
                        psb = psumC.tile([P, T], F32, tag="bps")
                        nc.tensor.matmul(psb, lhsT=b2_sb[:, ts(j, P)],
                                         rhs=routeT[:, :],
                                         start=True, stop=True)
                        nc.vector.tensor_copy(out_sb[:, j, :], psb)
                    bsrc = bgb if j < JH else bfb
                    tmp = apool.tile([P, T], F32, tag="acc")
                    nc.vector.tensor_mul(tmp, pso, bsrc)
                    nc.vector.tensor_add(out_sb[:, j, :], out_sb[:, j, :], tmp)
                    if e == E - 1:
                        # final value for this d-tile: stream it out now,
                        # split in two so the last store's transfer halves
                        yt = y0 if j < JH else y1
                        jj = j if j < JH else j - JH
                        for h2 in range(2):
                            nc.scalar.dma_start(
                                yt[ts(jj, P), ts(h2, T // 2)],
                                out_sb[:, j, ts(h2, T // 2)])

    nc.compile()
    return nc


_NC_CACHE = {}


def _get_nc():
    if "nc" not in _NC_CACHE:
        _NC_CACHE["nc"] = _build_moe(T, D, E)
    return _NC_CACHE["nc"]


def _fingerprint(*arrays):
    parts = []
    for a in arrays:
        a = np.asarray(a)
        flat = a.reshape(-1)
        step = max(1, flat.size // 64)
        parts.append((id(a), a.shape, flat[::step][:64].tobytes()))
    return hash(tuple((i, s, b) for i, s, b in parts))


def _prep_shared(Wg, bg, Wf, bf, W1, b1, W2, b2, weight):
    KT = D // P
    HKo = H // P
    f32 = np.float32
    bf16 = ml_dtypes.bfloat16
    return {
        "w1r": np.ascontiguousarray(
            W1.reshape(E, KT, P, KT, P).transpose(0, 3, 2, 1, 4)).astype(bf16),
        "w2r": np.ascontiguousarray(
            W2.reshape(E, KT, P, KT, P).transpose(0, 3, 2, 1, 4)).astype(bf16),
        "wg": np.ascontiguousarray(
            Wg.reshape(HKo, P, E).transpose(1, 0, 2)).astype(f32, copy=False),
        "wf": np.ascontiguousarray(
            Wf.reshape(HKo, P, E).transpose(1, 0, 2)).astype(f32, copy=False),
        "bgt": np.ascontiguousarray(np.asarray(bg, f32).reshape(1, E)),
        "bft": np.ascontiguousarray(np.asarray(bf, f32).reshape(1, E)),
        "b1r": np.ascontiguousarray(
            b1.reshape(E, KT, P).transpose(2, 0, 1)).astype(f32, copy=False),
        "b2t": np.ascontiguousarray(np.asarray(b2, f32)).astype(bf16),
        "wv": np.ascontiguousarray(np.asarray(weight, f32).reshape(1, 2)),
    }


def kernel(vector, Wg, bg, Wf, bf, W1, b1, W2, b2, weight, top_k):
    """Full inputs in, full output out (tuple (out0, out1), matching the
    reference)."""
    global LAST_EXEC_TIME_NS
    assert int(top_k) == TOPK, f"kernel compiled for top_k={TOPK}"
    vector = np.asarray(vector, np.float32)
    assert vector.shape == (B, S, D), vector.shape

    nc = _get_nc()
    fp = _fingerprint(Wg, bg, Wf, bf, W1, b1, W2, b2, weight)
    if _NC_CACHE.get("shared_fp") != fp:
        _NC_CACHE["shared"] = _prep_shared(
            np.asarray(Wg, np.float32), bg, np.asarray(Wf, np.float32), bf,
            np.asarray(W1, np.float32), np.asarray(b1, np.float32),
            np.asarray(W2, np.float32), np.asarray(b2, np.float32), weight)
        _NC_CACHE["shared_fp"] = fp
    shared = _NC_CACHE["shared"]

    tokens = vector.reshape(B * S, D)
    in_maps = []
    for c in range(NCORES):
        m = dict(shared)
        xtc = np.ascontiguousarray(tokens[c * T:(c + 1) * T].T)
        m["xt"] = xtc
        m["xb"] = xtc.astype(ml_dtypes.bfloat16)
        in_maps.append(m)

    trace = bool(os.environ.get("MOE_TRACE"))
    res = run_bass_kernel_spmd(nc, in_maps, core_ids=list(range(NCORES)),
                               trace=trace)
    if trace:
        LAST_EXEC_TIME_NS = res.exec_time_ns

    y0 = np.stack([res.results[c]["y0"].T for c in range(NCORES)])
    y1 = np.stack([res.results[c]["y1"].T for c in range(NCORES)])
    out0 = np.ascontiguousarray(y0.reshape(B, S, H))
    out1 = np.ascontiguousarray(y1.reshape(B, S, H))
    return (out0, out1)
